# revision 15
# baseline (speedup 1.0000x reference)
"""EpisodicMemory cross-attention kernel for Trainium2 (8 NeuronCores).

Per-core computation (batch example b on core b, pure data parallel):
    mem_slots = x[::8]                      # (S, D)   S = T/8
    mem_k     = mem_slots @ W_comp          # (S, M)
    Q         = x @ W_q                     # (T, M)
    K         = mem_k @ W_k                 # (S, M)
    scores    = Q @ K.T / sqrt(M)           # (T, S)
    attn      = softmax(scores, -1)
    out       = (attn @ mem_slots) @ W_comp @ W_v * sigmoid(x @ W_gate + b)

Key restructurings vs the reference:
  * (attn @ mem_slots) @ W_comp == attn @ mem_k  — reuses the projected
    memory keys and removes the two largest GEMMs (saves ~40% of FLOPs).
  * scores stay natural [t, s]; softmax needs no max-subtraction
    (|scores| < 8 for unit-scale inputs) and the denominator comes free
    from the Exp activation's accum_out.
  * gate + 1/rowsum are folded into a single per-token scale applied to
    the attention weights before the context GEMM, so all downstream
    evacuations are plain copies.
  * all GEMMs run as float32r (full-rate PE) with fp32 data; f32r pays
    a full LDWEIGHTS per matmul, so tokens are processed in 512-wide
    pairs to maximize moving-operand size per weight load.
"""

import math
import os
import sys
from contextlib import ExitStack

import numpy as np

for _p in ("/opt/trn_rl_repo", "/root/.axon_site/_ro/trn_rl_repo"):
    if os.path.isdir(_p) and _p not in sys.path:
        sys.path.append(_p)

import concourse.bacc as bacc
import concourse.mybir as mybir
import concourse.tile as tile
from concourse.masks import make_identity

P = 128
D = 2048
M = 512
MEM_FREQ = 8
F32 = mybir.dt.float32
F32R = mybir.dt.float32r
ALU = mybir.AluOpType
ACTF = mybir.ActivationFunctionType


def _r(ap):
    return ap.bitcast(F32R)


def build_nc(T=4096):
    """Build the single-core Bass program for one (T, D) example."""
    S = T // MEM_FREQ
    SB = S // P        # s-blocks
    DB = D // P        # 16
    MB = M // P        # 4
    PTC = 512          # tokens per pair-chunk
    PTB = PTC // P     # 4 t-blocks per pair
    HTB = 2            # t-blocks per half (x DMA granularity)
    NP = T // PTC      # pairs
    NDSL = D // 512
    scale = 1.0 / math.sqrt(M)

    nc = bacc.Bacc("TRN2", target_bir_lowering=False)
    x_d = nc.dram_tensor("x", [T, D], F32, kind="ExternalInput").ap()
    wcomp_d = nc.dram_tensor("W_comp", [D, M], F32, kind="ExternalInput").ap()
    wq_d = nc.dram_tensor("W_q", [D, M], F32, kind="ExternalInput").ap()
    wk_d = nc.dram_tensor("W_k", [M, M], F32, kind="ExternalInput").ap()
    wv_d = nc.dram_tensor("W_v", [M, D], F32, kind="ExternalInput").ap()
    wg_d = nc.dram_tensor("W_gate", [D, 1], F32, kind="ExternalInput").ap()
    bg_d = nc.dram_tensor("b_gate", [1], F32, kind="ExternalInput").ap()
    out_d = nc.dram_tensor("out", [T, D], F32, kind="ExternalOutput").ap()

    with tile.TileContext(nc) as tc, ExitStack() as ctx:
        const = ctx.enter_context(tc.tile_pool(name="const", bufs=1))
        # PSUM pools span both passes so pass-2 PE work can overlap pass-1.
        ps_tr = ctx.enter_context(tc.tile_pool(name="pstr", bufs=2, space="PSUM"))
        ps_big = ctx.enter_context(tc.tile_pool(name="psbig", bufs=1, space="PSUM"))
        ps_so = ctx.enter_context(tc.tile_pool(name="psso", bufs=2, space="PSUM"))

        wq_sb = const.tile([P, DB, M], F32R)      # [d_in, d_blk, m]
        wv_sb = const.tile([P, MB, D], F32R)      # [m_in, m_blk, d]
        ident = const.tile([P, P], F32)
        make_identity(nc, ident)
        wg_bcast = const.tile([P, D], F32)        # W_gate replicated on all partitions
        nc.sync.dma_start(
            wg_bcast, wg_d.rearrange("d o -> o d").partition_broadcast(P)
        )
        bg_bcast = const.tile([P, 1], F32)
        nc.sync.dma_start(bg_bcast, bg_d[None, :].partition_broadcast(P))
        kt_sb = const.tile([P, MB, S], F32R)       # K^T: [m'_in, m'_blk, s]
        memk_sb = const.tile([P, SB, M], F32R)     # mem_k: [s_in, s_blk, m]

        # ---------------- pass 1: mem_k and K^T ----------------
        with (
            tc.tile_pool(name="p1", bufs=2) as p1,
            tc.tile_pool(name="p1c", bufs=1) as p1c,
        ):
            wc_sb = p1c.tile([P, DB, M], F32R)
            wk_sb = p1c.tile([P, MB, M], F32R)
            # stage fp32 weights through a transient pool; the engine copy
            # rounds to the f32r storage format walrus requires for operands
            # of f32r matmuls
            with tc.tile_pool(name="stage", bufs=2) as stage:
                STW = 2048  # staging slice: 8KB/partition
                for dst, src_ap, eng in (
                    (wq_sb, wq_d.rearrange("(o p) m -> p o m", p=P), "v"),
                    (wv_sb, wv_d.rearrange("(o p) d -> p o d", p=P), "s"),
                    (wc_sb, wcomp_d.rearrange("(o p) m -> p o m", p=P), "v"),
                    (wk_sb, wk_d.rearrange("(o p) m -> p o m", p=P), "s"),
                ):
                    nblk, blk = dst.shape[1], dst.shape[2]
                    grp = max(1, STW // blk)
                    for o in range(0, nblk, grp):
                        g = min(grp, nblk - o)
                        st = stage.tile([P, STW], F32, tag="st")
                        flat = st[:, : g * blk].rearrange("p (a b) -> p a b", a=g)
                        nc.sync.dma_start(flat, src_ap[:, o : o + g, :])
                        if eng == "v":
                            nc.vector.tensor_copy(dst[:, o : o + g, :], flat)
                        else:
                            nc.scalar.copy(dst[:, o : o + g, :], flat)

            msT = p1c.tile([P, DB, S], F32R)       # mem_slots^T: [d_in, d_blk, s]
            memkT = p1c.tile([P, MB, S], F32R)     # mem_k^T: [m_in, m_blk, s]

            xr = x_d.rearrange("(sb p f) d -> p sb f d", p=P, f=MEM_FREQ)
            for sb in range(SB):
                ms_nat = p1.tile([P, D], F32, tag="ms")
                nc.sync.dma_start(ms_nat, xr[:, sb, 0, :])
                for db2 in range(0, DB, 4):
                    ps = ps_tr.tile([P, 4 * P], F32, tag="tr")
                    for j in range(4):
                        nc.tensor.transpose(
                            ps[:, j * P : (j + 1) * P],
                            ms_nat[:, (db2 + j) * P : (db2 + j + 1) * P],
                            ident,
                        )
                    eng = nc.vector if (db2 // 4) % 2 == 0 else nc.scalar
                    (eng.tensor_copy if eng is nc.vector else nc.scalar.copy)(
                        msT[:, db2 : db2 + 4, sb * P : (sb + 1) * P],
                        ps.rearrange("p (a q) -> p a q", a=4),
                    )
            # mem_k (natural [s, m])
            for sb in range(SB):
                ps = ps_so.tile([P, M], F32, tag="so")
                for db in range(DB):
                    nc.tensor.matmul(
                        ps,
                        msT[:, db, sb * P : (sb + 1) * P],
                        wc_sb[:, db, :],
                        start=(db == 0),
                        stop=(db == DB - 1),
                    )
                nc.vector.tensor_copy(memk_sb[:, sb, :], ps)
            # mem_k^T via PE transposes of mem_k blocks (f32 path)
            for mb in range(MB):
                ps = ps_tr.tile([P, SB * P], F32, tag="tr")
                for sb in range(SB):
                    nc.tensor.transpose(
                        ps[:, sb * P : (sb + 1) * P],
                        memk_sb.bitcast(F32)[:, sb, mb * P : (mb + 1) * P],
                        ident,
                    )
                nc.scalar.copy(memkT[:, mb, :], ps)
            # K^T = W_k^T @ mem_k^T
            for mb2 in range(MB):
                ps = ps_so.tile([P, S], F32, tag="so")
                for mb in range(MB):
                    nc.tensor.matmul(
                        ps,
                        wk_sb[:, mb, mb2 * P : (mb2 + 1) * P],
                        memkT[:, mb, :],
                        start=(mb == 0),
                        stop=(mb == MB - 1),
                    )
                nc.vector.tensor_copy(kt_sb[:, mb2, :], ps)

        # ---------------- pass 2: main loop over 512-token pairs ----------------
        with (
            tc.tile_pool(name="xin", bufs=2) as xin_pool,
            tc.tile_pool(name="xt", bufs=1) as xt_pool,
            tc.tile_pool(name="mid", bufs=1) as mid_pool,
            tc.tile_pool(name="sml", bufs=2) as sml_pool,
            tc.tile_pool(name="outp", bufs=2) as out_pool,
            tc.tile_pool(name="scr", bufs=1) as scr_pool,
        ):
            xr2 = x_d.rearrange("(c h tb p) d -> c h p tb d", p=P, tb=HTB, h=2)
            our = out_d.rearrange("(c tb p) d -> c tb p d", p=P, tb=PTB)
            for c in range(NP):
                # x^T for the whole pair, via PE transposes (packed 4/bank)
                xT = xt_pool.tile([P, DB, PTC], F32R, tag="xT")
                halves = []
                for h in range(2):
                    x_nat = xin_pool.tile([P, HTB, D], F32, tag="x")
                    nc.sync.dma_start(x_nat, xr2[c, h])
                    halves.append(x_nat)
                    for db2 in range(0, DB, 2):
                        ps = ps_tr.tile([P, 2 * HTB * P], F32, tag="tr")
                        k = 0
                        for dbo in range(2):
                            for tb in range(HTB):
                                nc.tensor.transpose(
                                    ps[:, k * P : (k + 1) * P],
                                    x_nat[
                                        :, tb, (db2 + dbo) * P : (db2 + dbo + 1) * P
                                    ],
                                    ident,
                                )
                                k += 1
                        dst = xT[:, db2 : db2 + 2, h * 256 : (h + 1) * 256]
                        src = ps.rearrange("p (a q) -> p a q", a=2)
                        if (db2 // 2) % 2 == 0:
                            nc.vector.tensor_copy(dst, src)
                        else:
                            nc.scalar.copy(dst, src)

                # gate: g = sigmoid(x @ W_gate + b); multiply on gpsimd,
                # free-dim reduction on DVE
                g_sb = sml_pool.tile([P, PTB], F32, tag="g")
                sc_v = sml_pool.tile([P, PTB], F32, tag="scv")
                rs_inv = sml_pool.tile([P, PTB], F32, tag="rsi")
                rows = sml_pool.tile([P, PTB], F32, tag="rows")
                for tb in range(PTB):
                    x_nat = halves[tb // HTB]
                    htb = tb % HTB
                    g4 = sml_pool.tile([P, 4], F32, tag="g4")
                    for gs in range(4):
                        scratch = scr_pool.tile([P, D // 4], F32, tag="scratch")
                        dsl = slice(gs * (D // 4), (gs + 1) * (D // 4))
                        nc.gpsimd.tensor_mul(
                            scratch, x_nat[:, htb, dsl], wg_bcast[:, dsl]
                        )
                        nc.vector.tensor_reduce(
                            g4[:, gs : gs + 1],
                            scratch,
                            axis=mybir.AxisListType.X,
                            op=ALU.add,
                        )
                    nc.vector.tensor_reduce(
                        g_sb[:, tb : tb + 1],
                        g4,
                        axis=mybir.AxisListType.X,
                        op=ALU.add,
                    )
                    nc.scalar.activation(
                        g_sb[:, tb : tb + 1],
                        g_sb[:, tb : tb + 1],
                        ACTF.Sigmoid,
                        bias=bg_bcast[:, 0:1],
                    )

                # Q^T[m, t_pair] accumulated over d-blocks (N=512 moving)
                qt_psum = ps_big.tile([P, MB * PTC], F32, tag="big")
                for mb in range(MB):
                    sl = qt_psum[:, mb * PTC : (mb + 1) * PTC]
                    for db in range(DB):
                        nc.tensor.matmul(
                            sl,
                            wq_sb[:, db, mb * P : (mb + 1) * P],
                            xT[:, db, :],
                            start=(db == 0),
                            stop=(db == DB - 1),
                        )
                qt_sb = mid_pool.tile([P, MB, PTC], F32R, tag="qt")
                for mb in range(MB):
                    dst = qt_sb[:, mb, :]
                    src = qt_psum[:, mb * PTC : (mb + 1) * PTC]
                    if mb % 2 == 0:
                        nc.vector.tensor_copy(dst, src)
                    else:
                        nc.scalar.copy(dst, src)

                # scores -> exp (+rowsum) -> fold g/rowsum into attn weights
                attn = mid_pool.tile([P, PTB, S], F32, tag="attn")
                for tb in range(PTB):
                    sc_ps = ps_so.tile([P, S], F32, tag="so")
                    for mb in range(MB):
                        nc.tensor.matmul(
                            sc_ps,
                            qt_sb[:, mb, tb * P : (tb + 1) * P],
                            kt_sb[:, mb, :],
                            start=(mb == 0),
                            stop=(mb == MB - 1),
                        )
                    nc.scalar.activation(
                        attn[:, tb, :],
                        sc_ps,
                        ACTF.Exp,
                        scale=scale,
                        accum_out=rows[:, tb : tb + 1],
                    )
                    nc.vector.reciprocal(rs_inv[:, tb : tb + 1], rows[:, tb : tb + 1])
                    nc.vector.tensor_mul(
                        sc_v[:, tb : tb + 1],
                        g_sb[:, tb : tb + 1],
                        rs_inv[:, tb : tb + 1],
                    )
                    nc.vector.tensor_scalar_mul(
                        attn[:, tb, :], attn[:, tb, :], sc_v[:, tb : tb + 1]
                    )

                # attn^T via PE transposes: one PSUM bank per s-block holds
                # all PTB t-blocks -> contiguous evac
                attnT = mid_pool.tile([P, SB, PTC], F32R, tag="attnT")
                for sb in range(SB):
                    ps = ps_tr.tile([P, PTB * P], F32, tag="tr")
                    for tb in range(PTB):
                        nc.tensor.transpose(
                            ps[:, tb * P : (tb + 1) * P],
                            attn[:, tb, sb * P : (sb + 1) * P],
                            ident,
                        )
                    if sb % 2 == 0:
                        nc.vector.tensor_copy(attnT[:, sb, :], ps)
                    else:
                        nc.scalar.copy(attnT[:, sb, :], ps)

                # ctxU^T[m, t_pair] = sum_s mem_k[s, m] * attnT[s, t]
                ctx_psum = ps_big.tile([P, MB * PTC], F32, tag="big")
                for mb in range(MB):
                    sl = ctx_psum[:, mb * PTC : (mb + 1) * PTC]
                    for sb in range(SB):
                        nc.tensor.matmul(
                            sl,
                            memk_sb[:, sb, mb * P : (mb + 1) * P],
                            attnT[:, sb, :],
                            start=(sb == 0),
                            stop=(sb == SB - 1),
                        )
                ctx_sb = mid_pool.tile([P, MB, PTC], F32R, tag="ctx")
                for mb in range(MB):
                    dst = ctx_sb[:, mb, :]
                    src = ctx_psum[:, mb * PTC : (mb + 1) * PTC]
                    if mb % 2 == 0:
                        nc.vector.tensor_copy(dst, src)
                    else:
                        nc.scalar.copy(dst, src)

                # out[t, d] = sum_m ctxU[t, m] * W_v[m, d]  (already gated+normalized)
                for tb in range(PTB):
                    out_sb = out_pool.tile([P, D], F32, tag="o")
                    for dsl in range(NDSL):
                        op = ps_so.tile([P, 512], F32, tag="so")
                        for mb in range(MB):
                            nc.tensor.matmul(
                                op,
                                ctx_sb[:, mb, tb * P : (tb + 1) * P],
                                wv_sb[:, mb, dsl * 512 : (dsl + 1) * 512],
                                start=(mb == 0),
                                stop=(mb == MB - 1),
                            )
                        nc.scalar.copy(out_sb[:, dsl * 512 : (dsl + 1) * 512], op)
                    nc.sync.dma_start(our[c, tb], out_sb)
    nc.compile()
    return nc


_CACHED = {}


def _get_nc(T=4096):
    if T not in _CACHED:
        _CACHED[T] = build_nc(T)
    return _CACHED[T]


def kernel(x, W_comp, W_q, W_k, W_v, W_gate, b_gate, _trace=False):
    from concourse.bass_utils import run_bass_kernel_spmd

    x = np.ascontiguousarray(np.asarray(x, dtype=np.float32))
    B, T, _ = x.shape
    weights = {
        "W_comp": np.ascontiguousarray(np.asarray(W_comp, dtype=np.float32)),
        "W_q": np.ascontiguousarray(np.asarray(W_q, dtype=np.float32)),
        "W_k": np.ascontiguousarray(np.asarray(W_k, dtype=np.float32)),
        "W_v": np.ascontiguousarray(np.asarray(W_v, dtype=np.float32)),
        "W_gate": np.ascontiguousarray(np.asarray(W_gate, dtype=np.float32)),
        "b_gate": np.ascontiguousarray(np.asarray(b_gate, dtype=np.float32)),
    }
    nc = _get_nc(T)
    core_ids = list(range(B))
    in_maps = [dict(weights, x=x[b]) for b in range(B)]
    res = run_bass_kernel_spmd(nc, in_maps, core_ids, trace=_trace)
    out = np.stack([res.results[b]["out"] for b in range(B)], axis=0)
    if _trace:
        return out, res
    return out


if __name__ == "__main__":
    nc = build_nc()
    n = sum(len(b.instructions) for b in nc.m.functions[0].blocks)
    print("built:", n, "instructions")


# revision 16
# speedup vs baseline: 1.1745x; 1.1745x over previous
"""EpisodicMemory cross-attention kernel for Trainium2 (8 NeuronCores).

Per-core computation (batch example b on core b, pure data parallel):
    mem_slots = x[::8]                      # (S, D)   S = T/8
    mem_k     = mem_slots @ W_comp          # (S, M)
    Q         = x @ W_q                     # (T, M)
    K         = mem_k @ W_k                 # (S, M)
    scores    = Q @ K.T / sqrt(M)           # (T, S)
    attn      = softmax(scores, -1)
    out       = (attn @ mem_slots) @ W_comp @ W_v * sigmoid(x @ W_gate + b)

Key restructurings vs the reference:
  * (attn @ mem_slots) @ W_comp == attn @ mem_k  — reuses the projected
    memory keys and removes the two largest GEMMs (saves ~40% of FLOPs).
  * scores stay natural [t, s]; softmax needs no max-subtraction
    (|scores| < 8 for unit-scale inputs) and the denominator comes free
    from the Exp activation's accum_out.
  * attention weights stay UNNORMALIZED through the context and output
    GEMMs; sigmoid(gate)/rowsum is applied as a per-partition scale on
    the output PSUM evacuation, keeping the exp -> transpose -> GEMM
    chain free of elementwise passes.
  * all GEMMs run as float32r (full-rate PE) with fp32 data; f32r pays
    a full LDWEIGHTS per matmul, so tokens are processed in 512-wide
    pairs to maximize moving-operand size per weight load.
"""

import math
import os
import sys
from contextlib import ExitStack

import numpy as np

for _p in ("/opt/trn_rl_repo", "/root/.axon_site/_ro/trn_rl_repo"):
    if os.path.isdir(_p) and _p not in sys.path:
        sys.path.append(_p)

import concourse.bacc as bacc
import concourse.mybir as mybir
import concourse.tile as tile
from concourse.masks import make_identity

P = 128
D = 2048
M = 512
MEM_FREQ = 8
F32 = mybir.dt.float32
F32R = mybir.dt.float32r
ALU = mybir.AluOpType
ACTF = mybir.ActivationFunctionType


def build_nc(T=4096):
    """Build the single-core Bass program for one (T, D) example."""
    S = T // MEM_FREQ
    SB = S // P        # s-blocks
    DB = D // P        # 16
    MB = M // P        # 4
    PTC = 512          # tokens per pair-chunk
    PTB = PTC // P     # 4 t-blocks per pair
    HTB = 2            # t-blocks per half (x DMA granularity)
    NP = T // PTC      # pairs
    NDSL = D // 512
    scale = 1.0 / math.sqrt(M)

    nc = bacc.Bacc("TRN2", target_bir_lowering=False)
    x_d = nc.dram_tensor("x", [T, D], F32, kind="ExternalInput").ap()
    wcomp_d = nc.dram_tensor("W_comp", [D, M], F32, kind="ExternalInput").ap()
    wq_d = nc.dram_tensor("W_q", [D, M], F32, kind="ExternalInput").ap()
    wk_d = nc.dram_tensor("W_k", [M, M], F32, kind="ExternalInput").ap()
    wv_d = nc.dram_tensor("W_v", [M, D], F32, kind="ExternalInput").ap()
    wg_d = nc.dram_tensor("W_gate", [D, 1], F32, kind="ExternalInput").ap()
    bg_d = nc.dram_tensor("b_gate", [1], F32, kind="ExternalInput").ap()
    out_d = nc.dram_tensor("out", [T, D], F32, kind="ExternalOutput").ap()

    with tile.TileContext(nc) as tc, ExitStack() as ctx:
        const = ctx.enter_context(tc.tile_pool(name="const", bufs=1))
        # PSUM pools span both passes so pass-2 PE work can overlap pass-1.
        ps_tr = ctx.enter_context(tc.tile_pool(name="pstr", bufs=2, space="PSUM"))
        ps_big = ctx.enter_context(tc.tile_pool(name="psbig", bufs=1, space="PSUM"))
        ps_so = ctx.enter_context(tc.tile_pool(name="psso", bufs=2, space="PSUM"))

        wq_sb = const.tile([P, DB, M], F32R)      # [d_in, d_blk, m]
        wv_sb = const.tile([P, MB, D], F32R)      # [m_in, m_blk, d]
        ident = const.tile([P, P], F32)
        make_identity(nc, ident)
        wg_bcast = const.tile([P, D], F32)        # W_gate replicated on all partitions
        nc.sync.dma_start(
            wg_bcast, wg_d.rearrange("d o -> o d").partition_broadcast(P)
        )
        bg_bcast = const.tile([P, 1], F32)
        nc.sync.dma_start(bg_bcast, bg_d[None, :].partition_broadcast(P))
        kt_sb = const.tile([P, MB, S], F32R)       # K^T: [m'_in, m'_blk, s]
        memk_sb = const.tile([P, SB, M], F32R)     # mem_k: [s_in, s_blk, m]

        def stage_weight(pool, dst, src_ap, round_eng, slice_w=2048):
            """DMA fp32 weight then round to f32r via an engine copy (the
            rounding producer walrus requires for f32r matmul operands)."""
            nblk, blk = dst.shape[1], dst.shape[2]
            grp = max(1, slice_w // blk)
            for o in range(0, nblk, grp):
                g = min(grp, nblk - o)
                st = pool.tile([P, slice_w], F32, tag="st", name="st")
                flat = st[:, : g * blk].rearrange("p (a b) -> p a b", a=g)
                nc.sync.dma_start(flat, src_ap[:, o : o + g, :])
                if round_eng == "v":
                    nc.vector.tensor_copy(dst[:, o : o + g, :], flat)
                elif round_eng == "s":
                    nc.scalar.copy(dst[:, o : o + g, :], flat)
                else:
                    nc.gpsimd.tensor_copy(dst[:, o : o + g, :], flat)

        # ---------------- pass 1: mem_k and K^T ----------------
        with (
            tc.tile_pool(name="p1", bufs=2) as p1,
            tc.tile_pool(name="p1t", bufs=4) as p1t,
            tc.tile_pool(name="p1c", bufs=1) as p1c,
            tc.tile_pool(name="stage", bufs=2) as stage,
        ):
            wc_sb = p1c.tile([P, DB, M], F32R)
            wk_sb = p1c.tile([P, MB, M], F32R)
            memkT = p1c.tile([P, MB, S], F32R)     # mem_k^T: [m_in, m_blk, s]

            # critical pass-1 weights first (DVE rounding, ahead of the
            # mem_slot evac copies in the DVE queue)
            stage_weight(stage, wc_sb, wcomp_d.rearrange("(o p) m -> p o m", p=P), "v")
            stage_weight(stage, wk_sb, wk_d.rearrange("(o p) m -> p o m", p=P), "v")

            xr = x_d.rearrange("(sb p f) d -> p sb f d", p=P, f=MEM_FREQ)
            # mem_slots^T is streamed: transpose one d-block, multiply it
            # into the mem_k accumulation, release the tile.
            for sb in range(SB):
                ms_nat = p1.tile([P, D], F32, tag="ms", name="ms")
                nc.sync.dma_start(ms_nat, xr[:, sb, 0, :])
                mk_ps = ps_so.tile([P, M], F32, tag="so", name="mkps")
                for db2 in range(0, DB, 4):
                    ps = ps_tr.tile([P, 4 * P], F32, tag="tr", name="trps")
                    for j in range(4):
                        nc.tensor.transpose(
                            ps[:, j * P : (j + 1) * P],
                            ms_nat[:, (db2 + j) * P : (db2 + j + 1) * P],
                            ident,
                        )
                    msT = p1t.tile([P, 4, P], F32R, tag="msT", name="msT")
                    if (db2 // 4) % 2 == 0:
                        nc.vector.tensor_copy(
                            msT, ps.rearrange("p (a q) -> p a q", a=4)
                        )
                    else:
                        nc.scalar.copy(msT, ps.rearrange("p (a q) -> p a q", a=4))
                    for j in range(4):
                        db = db2 + j
                        nc.tensor.matmul(
                            mk_ps,
                            msT[:, j, :],
                            wc_sb[:, db, :],
                            start=(db == 0),
                            stop=(db == DB - 1),
                        )
                nc.vector.tensor_copy(memk_sb[:, sb, :], mk_ps)
            # mem_k^T via PE transposes of mem_k blocks (f32 path)
            for mb in range(MB):
                ps = ps_tr.tile([P, SB * P], F32, tag="tr", name="mktps")
                for sb in range(SB):
                    nc.tensor.transpose(
                        ps[:, sb * P : (sb + 1) * P],
                        memk_sb.bitcast(F32)[:, sb, mb * P : (mb + 1) * P],
                        ident,
                    )
                nc.scalar.copy(memkT[:, mb, :], ps)
            # K^T = W_k^T @ mem_k^T
            for mb2 in range(MB):
                ps = ps_so.tile([P, S], F32, tag="so", name="ktps")
                for mb in range(MB):
                    nc.tensor.matmul(
                        ps,
                        wk_sb[:, mb, mb2 * P : (mb2 + 1) * P],
                        memkT[:, mb, :],
                        start=(mb == 0),
                        stop=(mb == MB - 1),
                    )
                nc.vector.tensor_copy(kt_sb[:, mb2, :], ps)

            # bulk weights for pass 2: DMA + gpsimd rounding in the
            # background while pass-1 PE work runs
            stage_weight(stage, wq_sb, wq_d.rearrange("(o p) m -> p o m", p=P), "g")
            stage_weight(stage, wv_sb, wv_d.rearrange("(o p) d -> p o d", p=P), "g")

        # ---------------- pass 2: main loop over 512-token pairs ----------------
        with (
            tc.tile_pool(name="xin", bufs=2) as xin_pool,
            tc.tile_pool(name="xt", bufs=1) as xt_pool,
            tc.tile_pool(name="mid", bufs=1) as mid_pool,
            tc.tile_pool(name="sml", bufs=2) as sml_pool,
            tc.tile_pool(name="outp", bufs=2) as out_pool,
            tc.tile_pool(name="scr", bufs=1) as scr_pool,
        ):
            xr2 = x_d.rearrange("(c h tb p) d -> c h p tb d", p=P, tb=HTB, h=2)
            our = out_d.rearrange("(c tb p) d -> c tb p d", p=P, tb=PTB)
            for c in range(NP):
                # x^T for the whole pair, via PE transposes (packed 4/bank)
                xT = xt_pool.tile([P, DB, PTC], F32R, tag="xT", name="xT")
                halves = []
                for h in range(2):
                    x_nat = xin_pool.tile([P, HTB, D], F32, tag="x", name="x")
                    nc.sync.dma_start(x_nat, xr2[c, h])
                    halves.append(x_nat)
                    for db2 in range(0, DB, 2):
                        ps = ps_tr.tile([P, 2 * HTB * P], F32, tag="tr", name="xtps")
                        k = 0
                        for dbo in range(2):
                            for tb in range(HTB):
                                nc.tensor.transpose(
                                    ps[:, k * P : (k + 1) * P],
                                    x_nat[
                                        :, tb, (db2 + dbo) * P : (db2 + dbo + 1) * P
                                    ],
                                    ident,
                                )
                                k += 1
                        dst = xT[:, db2 : db2 + 2, h * 256 : (h + 1) * 256]
                        src = ps.rearrange("p (a q) -> p a q", a=2)
                        if (db2 // 2) % 2 == 0:
                            nc.vector.tensor_copy(dst, src)
                        else:
                            nc.scalar.copy(dst, src)

                # Q^T[m, t_pair] accumulated over d-blocks (N=512 moving)
                qt_psum = ps_big.tile([P, MB * PTC], F32, tag="big", name="qtps")
                for mb in range(MB):
                    sl = qt_psum[:, mb * PTC : (mb + 1) * PTC]
                    for db in range(DB):
                        nc.tensor.matmul(
                            sl,
                            wq_sb[:, db, mb * P : (mb + 1) * P],
                            xT[:, db, :],
                            start=(db == 0),
                            stop=(db == DB - 1),
                        )
                qt_sb = mid_pool.tile([P, MB, PTC], F32R, tag="qt", name="qt")
                for mb in range(MB):
                    dst = qt_sb[:, mb, :]
                    src = qt_psum[:, mb * PTC : (mb + 1) * PTC]
                    if mb % 2 == 0:
                        nc.vector.tensor_copy(dst, src)
                    else:
                        nc.scalar.copy(dst, src)

                # scores -> exp (+row sums); attn stays UNNORMALIZED.
                # Each t-block's transposes follow its exp immediately.
                rows = sml_pool.tile([P, PTB], F32, tag="rows", name="rows")
                attn = mid_pool.tile([P, PTB, S], F32, tag="attn", name="attn")
                attnT = mid_pool.tile([P, SB, PTC], F32R, tag="attnT", name="attnT")
                for tb in range(PTB):
                    sc_ps = ps_so.tile([P, S], F32, tag="so", name="scps")
                    for mb in range(MB):
                        nc.tensor.matmul(
                            sc_ps,
                            qt_sb[:, mb, tb * P : (tb + 1) * P],
                            kt_sb[:, mb, :],
                            start=(mb == 0),
                            stop=(mb == MB - 1),
                        )
                    nc.scalar.activation(
                        attn[:, tb, :],
                        sc_ps,
                        ACTF.Exp,
                        scale=scale,
                        accum_out=rows[:, tb : tb + 1],
                    )
                    ps = ps_tr.tile([P, SB * P], F32, tag="tr", name="atps")
                    for sb in range(SB):
                        nc.tensor.transpose(
                            ps[:, sb * P : (sb + 1) * P],
                            attn[:, tb, sb * P : (sb + 1) * P],
                            ident,
                        )
                    dst = attnT[:, :, tb * P : (tb + 1) * P]
                    src = ps.rearrange("p (a q) -> p a q", a=SB)
                    if tb % 2 == 0:
                        nc.vector.tensor_copy(dst, src)
                    else:
                        nc.scalar.copy(dst, src)

                # ctxU^T[m, t_pair] = sum_s mem_k[s, m] * attnT[s, t]
                ctx_psum = ps_big.tile([P, MB * PTC], F32, tag="big", name="ctxps")
                for mb in range(MB):
                    sl = ctx_psum[:, mb * PTC : (mb + 1) * PTC]
                    for sb in range(SB):
                        nc.tensor.matmul(
                            sl,
                            memk_sb[:, sb, mb * P : (mb + 1) * P],
                            attnT[:, sb, :],
                            start=(sb == 0),
                            stop=(sb == SB - 1),
                        )
                ctx_sb = mid_pool.tile([P, MB, PTC], F32R, tag="ctx", name="ctx")
                for mb in range(MB):
                    dst = ctx_sb[:, mb, :]
                    src = ctx_psum[:, mb * PTC : (mb + 1) * PTC]
                    if mb % 2 == 0:
                        nc.vector.tensor_copy(dst, src)
                    else:
                        nc.scalar.copy(dst, src)

                # gate (off the critical path — needed only by the out-evac
                # scale): g = sigmoid(x @ W_gate + b), then sc = g / rowsum
                g_sb = sml_pool.tile([P, PTB], F32, tag="g", name="g")
                sc_v = sml_pool.tile([P, PTB], F32, tag="scv", name="scv")
                rs_inv = sml_pool.tile([P, PTB], F32, tag="rsi", name="rsi")
                for tb in range(PTB):
                    x_nat = halves[tb // HTB]
                    htb = tb % HTB
                    g4 = sml_pool.tile([P, 4], F32, tag="g4", name="g4")
                    for gs in range(4):
                        scratch = scr_pool.tile(
                            [P, D // 4], F32, tag="scratch", name="scratch"
                        )
                        dsl = slice(gs * (D // 4), (gs + 1) * (D // 4))
                        nc.vector.tensor_mul(
                            scratch, x_nat[:, htb, dsl], wg_bcast[:, dsl]
                        )
                        nc.vector.tensor_reduce(
                            g4[:, gs : gs + 1],
                            scratch,
                            axis=mybir.AxisListType.X,
                            op=ALU.add,
                        )
                    nc.vector.tensor_reduce(
                        g_sb[:, tb : tb + 1],
                        g4,
                        axis=mybir.AxisListType.X,
                        op=ALU.add,
                    )
                    nc.scalar.activation(
                        g_sb[:, tb : tb + 1],
                        g_sb[:, tb : tb + 1],
                        ACTF.Sigmoid,
                        bias=bg_bcast[:, 0:1],
                    )
                    nc.vector.reciprocal(rs_inv[:, tb : tb + 1], rows[:, tb : tb + 1])
                    nc.vector.tensor_mul(
                        sc_v[:, tb : tb + 1],
                        g_sb[:, tb : tb + 1],
                        rs_inv[:, tb : tb + 1],
                    )

                # out[t, d] = sum_m ctxU[t, m] * W_v[m, d]; the ACT evac
                # applies the g/rowsum per-token scale
                for tb in range(PTB):
                    out_sb = out_pool.tile([P, D], F32, tag="o", name="o")
                    for dsl in range(NDSL):
                        op = ps_so.tile([P, 512], F32, tag="so", name="ops")
                        for mb in range(MB):
                            nc.tensor.matmul(
                                op,
                                ctx_sb[:, mb, tb * P : (tb + 1) * P],
                                wv_sb[:, mb, dsl * 512 : (dsl + 1) * 512],
                                start=(mb == 0),
                                stop=(mb == MB - 1),
                            )
                        nc.scalar.mul(
                            out_sb[:, dsl * 512 : (dsl + 1) * 512],
                            op,
                            sc_v[:, tb : tb + 1],
                        )
                    nc.sync.dma_start(our[c, tb], out_sb)
    nc.compile()
    return nc


_CACHED = {}


def _get_nc(T=4096):
    if T not in _CACHED:
        _CACHED[T] = build_nc(T)
    return _CACHED[T]


def kernel(x, W_comp, W_q, W_k, W_v, W_gate, b_gate, _trace=False):
    from concourse.bass_utils import run_bass_kernel_spmd

    x = np.ascontiguousarray(np.asarray(x, dtype=np.float32))
    B, T, _ = x.shape
    weights = {
        "W_comp": np.ascontiguousarray(np.asarray(W_comp, dtype=np.float32)),
        "W_q": np.ascontiguousarray(np.asarray(W_q, dtype=np.float32)),
        "W_k": np.ascontiguousarray(np.asarray(W_k, dtype=np.float32)),
        "W_v": np.ascontiguousarray(np.asarray(W_v, dtype=np.float32)),
        "W_gate": np.ascontiguousarray(np.asarray(W_gate, dtype=np.float32)),
        "b_gate": np.ascontiguousarray(np.asarray(b_gate, dtype=np.float32)),
    }
    nc = _get_nc(T)
    core_ids = list(range(B))
    in_maps = [dict(weights, x=x[b]) for b in range(B)]
    res = run_bass_kernel_spmd(nc, in_maps, core_ids, trace=_trace)
    out = np.stack([res.results[b]["out"] for b in range(B)], axis=0)
    if _trace:
        return out, res
    return out


if __name__ == "__main__":
    nc = build_nc()
    n = sum(len(b.instructions) for b in nc.m.functions[0].blocks)
    print("built:", n, "instructions")
